# revision 12
# baseline (speedup 1.0000x reference)
"""BinaryLeNet5 forward on 8 TRN2 NeuronCores, pure data parallel (1024 imgs/core).

Mapping summary (per core):
  conv1: kh-accumulated banded-Toeplitz matmuls, split into a 2-level precision
         ladder that is ~f32-exact but runs the PE at 2 cycles/row (vs 4 for
         native f32 matmuls):
           hi pass : fp16(x) against fp16 Toeplitz (+-1 exact), 1 cyc/row
           lo pass : fp16((x-hi)*2^11) against the Toeplitz scaled +-2^-11
                     (exact fp16 normals), 1 cyc/row
         All products are exact; PSUM accumulates in f32 -> ~22 effective
         mantissa bits on x, which simulation shows gives 0 mismatches.
  pool+sign: maxpool pairs are (a) psum free-dim pairs (ho parity) and (b) two
         PSUM tiles (wo parity) -> dense DVE maxes, then ACT Sign with f32
         per-partition bias. Sign output written twice into an fp8 pair layout
         x2dr[p, s, h] (slot s holds row h+s) so conv2 can contract kh-pairs.
  conv2: inputs/weights exactly +-1 in e4m3 -> kh taps {0,1},{2,3} are two
         DoubleRow matmuls (pair axis = kh tap), tap 4 a plain fp8 matmul.
  fc1:   same DoubleRow kh-pairing over the 5 h-blocks of the 400-dim input.
  fc2/fc3: tiny, bf16 (+-1 exact), f32 PSUM, biases in f32 via ACT bias.
  hardtanh drops out everywhere: sign(clip(x)) == sign(x), max(clip) == clip(max).

DMA: conv1 weights first, then the x streams split across both HWDGE rings
(sync: fp16 hi stream, scalar: fp16 scaled-residual stream), then the
conv2/fc weights, so compute starts ~14us in.

Output written as [10, 1024] per core, transposed/stacked on host.
"""

import os
import sys

import numpy as np

sys.path.insert(0, "/opt/trn_rl_repo")

import ml_dtypes  # noqa: E402

BF16 = ml_dtypes.bfloat16
F8E4 = ml_dtypes.float8_e4m3
F8E5 = ml_dtypes.float8_e5m2

B = 8192
NCORES = 8
N = B // NCORES  # 1024 images per core
NBLK = 2  # n blocks of 512 columns
NB = N // NBLK  # 512
HCH = 2  # h rows per x sbuf tile


def _binarize(w):
    return np.where(w >= 0, 1.0, -1.0).astype(np.float32)


def _build_t1h(w1):
    # t1[c*32+wi, kh*168 + par*84 + wo2*6 + o] = w1b[o,c,kh,kw]
    #   wo = 2*wo2 + par (par = wo parity), kw = wi - wo, valid 0<=kw<5
    w1b = _binarize(w1)  # [6,3,5,5]
    t1 = np.zeros((96, 5 * 168), np.float32)
    for kh in range(5):
        for par in range(2):
            for wo2 in range(14):
                wo = 2 * wo2 + par
                for o in range(6):
                    col = kh * 168 + par * 84 + wo2 * 6 + o
                    for c in range(3):
                        for kw in range(5):
                            wi = wo + kw
                            if wi < 32:
                                t1[c * 32 + wi, col] = w1b[o, c, kh, kw]
    return t1.astype(np.float16)


def _build_t1l(w1):
    # lo-pass weights: the fp16 Toeplitz scaled by 2^-11 (exact fp16 normals).
    return (_build_t1h(w1).astype(np.float32) * 2.0**-11).astype(np.float16)


def _build_t2(w2):
    # DR pairs: t2dr[w2*6+c, s, (ks*2+wop)*96 + wo2*16+o] = w2b[o,c,2ks+s,kw]
    # tap4:     t24 [w2*6+c, wop*96 + wo2*16+o] = w2b[o,c,4,kw]
    w2b = _binarize(w2)  # [16,6,5,5]
    t2dr = np.zeros((84, 2, 4 * 96), np.float32)
    t24 = np.zeros((84, 2 * 96), np.float32)
    for wop in range(2):
        for wo2 in range(5):
            wo = 2 * wo2 + wop
            for o in range(16):
                for c in range(6):
                    for kw in range(5):
                        w2i = wo + kw
                        if w2i >= 14:
                            continue
                        row = w2i * 6 + c
                        for ks in range(2):
                            for s in range(2):
                                t2dr[row, s, (ks * 2 + wop) * 96 + wo2 * 16 + o] = w2b[
                                    o, c, 2 * ks + s, kw
                                ]
                        t24[row, wop * 96 + wo2 * 16 + o] = w2b[o, c, 4, kw]
    return (
        np.ascontiguousarray(t2dr.reshape(84, 2 * 384)).astype(F8E4),
        t24.astype(F8E4),
    )


def _build_f1(wf1):
    # DR pairs: f1dr[w*16+o, s, h5p*128 + f] = wf1b[f, o*25+(2*h5p+s)*5+w]
    # tap4:     f14 [w*16+o, f] = wf1b[f, o*25+20+w]
    wf1b = _binarize(wf1)  # [120, 400]
    f1dr = np.zeros((80, 2, 2 * 128), np.float32)
    f14 = np.zeros((80, 120), np.float32)
    for w in range(5):
        for o in range(16):
            row = w * 16 + o
            for h5p in range(2):
                for s in range(2):
                    f1dr[row, s, h5p * 128 : h5p * 128 + 120] = wf1b[
                        :, o * 25 + (2 * h5p + s) * 5 + w
                    ]
            f14[row, :] = wf1b[:, o * 25 + 20 + w]
    return (
        np.ascontiguousarray(f1dr.reshape(80, 2 * 256)).astype(F8E4),
        f14.astype(F8E4),
    )


_CACHE = {}


def _get_nc():
    if "nc" in _CACHE:
        return _CACHE["nc"]
    import concourse.bacc as bacc
    import concourse.mybir as mybir
    import concourse.tile as tile

    f32 = mybir.dt.float32
    f16 = mybir.dt.float16
    bf16 = mybir.dt.bfloat16
    f8e4 = mybir.dt.float8e4
    f8e5 = mybir.dt.float8e5
    DR = mybir.MatmulPerfMode.DoubleRow

    nc = bacc.Bacc()
    xh_d = nc.dram_tensor("xh", [96, 32 * N], f16, kind="ExternalInput")
    xl_d = nc.dram_tensor("xl", [96, 32 * N], f16, kind="ExternalInput")
    t1h_d = nc.dram_tensor("t1h", [96, 840], f16, kind="ExternalInput")
    t1l_d = nc.dram_tensor("t1l", [96, 840], f16, kind="ExternalInput")
    t2dr_d = nc.dram_tensor("t2dr", [84, 768], f8e4, kind="ExternalInput")
    t24_d = nc.dram_tensor("t24", [84, 192], f8e4, kind="ExternalInput")
    f1dr_d = nc.dram_tensor("f1dr", [80, 512], f8e4, kind="ExternalInput")
    f14_d = nc.dram_tensor("f14", [80, 120], f8e4, kind="ExternalInput")
    f2_d = nc.dram_tensor("f2", [120, 84], bf16, kind="ExternalInput")
    f3_d = nc.dram_tensor("f3", [84, 10], bf16, kind="ExternalInput")
    b1_d = nc.dram_tensor("b1v", [84, 1], f32, kind="ExternalInput")
    b2_d = nc.dram_tensor("b2v", [80, 1], f32, kind="ExternalInput")
    bf1_d = nc.dram_tensor("bf1v", [120, 1], f32, kind="ExternalInput")
    bf2_d = nc.dram_tensor("bf2v", [84, 1], f32, kind="ExternalInput")
    bf3_d = nc.dram_tensor("bf3v", [10, 1], f32, kind="ExternalInput")
    out_d = nc.dram_tensor("out", [10, N], f32, kind="ExternalOutput")

    with tile.TileContext(nc) as tc:
        with (
            tc.tile_pool(name="xtp", bufs=1) as xtp,
            tc.tile_pool(name="wts", bufs=1) as wts,
            tc.tile_pool(name="acts", bufs=1) as acts,
            tc.tile_pool(name="ev", bufs=3) as ev,
            tc.tile_pool(name="ps", bufs=4, space="PSUM") as ps,
        ):
            # ---- DMA issue order: conv1 weights, then the x streams
            # interleaved across BOTH HWDGE rings (hi/lo tiles alternate
            # rings) so the early rows of both streams arrive in parallel,
            # then the weights that are only needed from conv2 onward.
            t1hs = wts.tile([96, 840], f16, tag="t1h")
            nc.sync.dma_start(out=t1hs, in_=t1h_d[:, :])
            t1ls = wts.tile([96, 840], f16, tag="t1l")
            nc.scalar.dma_start(out=t1ls, in_=t1l_d[:, :])
            b1s = wts.tile([84, 1], f32, tag="b1")
            nc.scalar.dma_start(out=b1s, in_=b1_d[:, :])

            xhs, xls = [], []
            for k in range(32 // HCH):
                sl = slice(k * HCH * N, (k + 1) * HCH * N)
                ring_h = nc.sync if k % 2 == 0 else nc.scalar
                ring_l = nc.scalar if k % 2 == 0 else nc.sync
                th = xtp.tile([96, HCH * N], f16, tag=f"xh{k}", name=f"xh{k}")
                ring_h.dma_start(out=th, in_=xh_d[:, sl])
                xhs.append(th)
                tl = xtp.tile([96, HCH * N], f16, tag=f"xl{k}", name=f"xl{k}")
                ring_l.dma_start(out=tl, in_=xl_d[:, sl])
                xls.append(tl)

            t2drs = wts.tile([84, 768], f8e4, tag="t2dr")
            nc.scalar.dma_start(out=t2drs, in_=t2dr_d[:, :])
            t24s = wts.tile([84, 192], f8e4, tag="t24")
            nc.scalar.dma_start(out=t24s, in_=t24_d[:, :])
            f1drs = wts.tile([80, 512], f8e4, tag="f1dr")
            nc.scalar.dma_start(out=f1drs, in_=f1dr_d[:, :])
            f14s = wts.tile([80, 120], f8e4, tag="f14")
            nc.scalar.dma_start(out=f14s, in_=f14_d[:, :])
            f2s = wts.tile([120, 84], bf16, tag="f2")
            nc.scalar.dma_start(out=f2s, in_=f2_d[:, :])
            f3s = wts.tile([84, 10], bf16, tag="f3")
            nc.scalar.dma_start(out=f3s, in_=f3_d[:, :])
            b2s = wts.tile([80, 1], f32, tag="b2")
            nc.scalar.dma_start(out=b2s, in_=b2_d[:, :])
            bf1s = wts.tile([120, 1], f32, tag="bf1")
            nc.scalar.dma_start(out=bf1s, in_=bf1_d[:, :])
            bf2s = wts.tile([84, 1], f32, tag="bf2")
            nc.scalar.dma_start(out=bf2s, in_=bf2_d[:, :])
            bf3s = wts.tile([10, 1], f32, tag="bf3")
            nc.scalar.dma_start(out=bf3s, in_=bf3_d[:, :])

            def xh_row(h, nb):  # [96, NB] fp16 slice for input row h
                return xhs[h // HCH][
                    :, (h % HCH) * N + nb * NB : (h % HCH) * N + nb * NB + NB
                ]

            def xl_row(h, nb):  # [96, NB] fp16 scaled-residual slice
                return xls[h // HCH][
                    :, (h % HCH) * N + nb * NB : (h % HCH) * N + nb * NB + NB
                ]

            # One consumer-engine 'touch' per DMA'd bias tile: the touch op
            # carries the DMA wait, so later ops on that engine need no extra
            # wait slot (TRN2 engine instructions have a single wait slot).
            tb1 = wts.tile([84, 1], f32, tag="tb1")
            nc.scalar.copy(tb1, b1s)
            tb2 = wts.tile([80, 1], f32, tag="tb2")
            nc.scalar.copy(tb2, b2s)
            tb3 = wts.tile([120, 1], f32, tag="tb3")
            nc.scalar.copy(tb3, bf1s)
            tb4 = wts.tile([84, 1], f32, tag="tb4")
            nc.scalar.copy(tb4, bf2s)
            tb5 = wts.tile([10, 1], f32, tag="tb5")
            nc.vector.tensor_copy(tb5, bf3s)

            x2dr = acts.tile([84, 2 * 14 * N], f8e4, tag="x2dr")
            x3dr = acts.tile([80, 2 * 3 * N], f8e4, tag="x3dr")
            x4 = acts.tile([120, N], bf16, tag="x4")
            x5 = acts.tile([84, N], bf16, tag="x5")
            outs = acts.tile([10, N], f32, tag="outs")
            x2v = x2dr.rearrange("p (s f) -> p s f", s=2)
            x3v = x3dr.rearrange("p (s f) -> p s f", s=2)
            t2drv = t2drs.rearrange("p (s f) -> p s f", s=2)
            f1drv = f1drs.rearrange("p (s f) -> p s f", s=2)

            # ---- block emitters ----
            # psum tile [84, 1024] = (hop 2) x (n 512) blocks; ho-pair pooled
            # in free dim by reduce_max; wo-pair = tensor_max of the two parity
            # chunks (same partitions). hi fp16 pass then fp16 scaled-residual
            # pass accumulate into the same PSUM region. hi and lo are emitted
            # separately so the start of conv1 can run hi-only while the lo
            # stream is still arriving.
            def conv1_hi(ho2, nb, p):
                for par in range(2):
                    for kh in range(5):
                        lhs = t1hs[:, kh * 168 + par * 84 : kh * 168 + par * 84 + 84]
                        for hop in range(2):
                            nc.tensor.matmul(
                                p[par][:, hop * NB : hop * NB + NB], lhs,
                                xh_row(2 * ho2 + hop + kh, nb),
                                start=(kh == 0), stop=False,
                            )

            def conv1_lo_pool(ho2, nb, p):
                for par in range(2):
                    for kh in range(5):
                        lhsl = t1ls[:, kh * 168 + par * 84 : kh * 168 + par * 84 + 84]
                        for hop in range(2):
                            nc.tensor.matmul(
                                p[par][:, hop * NB : hop * NB + NB], lhsl,
                                xl_row(2 * ho2 + hop + kh, nb),
                                start=False, stop=(kh == 4),
                            )
                e1 = []
                for par in range(2):
                    e = ev.tile([96, NB], f32, tag="ea", name="e1")[0:84]
                    nc.vector.reduce_max(e, p[par].rearrange("q (h n) -> q n h", h=2), axis=mybir.AxisListType.X)
                    e1.append(e)
                e2 = ev.tile([96, NB], f32, tag="ec", name="e2")[0:84]
                nc.vector.tensor_max(e2, e1[0], e1[1])
                s0 = x2v[:, 0, ho2 * N + nb * NB : ho2 * N + nb * NB + NB]
                nc.scalar.sign(s0, e2, bias=b1s)
                # duplicate into slot 1 at row ho2-1 (read by conv2 only up to
                # row 11, so skip the never-read ho2-1 >= 12 copies); DVE copy
                # keeps the ACT queue free for the critical sign chain.
                if 0 < ho2 <= 12:
                    nc.vector.tensor_copy(
                        x2v[:, 1, (ho2 - 1) * N + nb * NB : (ho2 - 1) * N + nb * NB + NB],
                        s0,
                    )

            def conv2_block(ho2, nb):
                # kh taps {0,1} and {2,3} via DoubleRow pair slots, tap 4 plain.
                p2 = [ps.tile([80, 2 * NB], f32, tag="ps", name="p2") for _ in range(2)]
                for wop in range(2):
                    for hop in range(2):
                        hb = 2 * ho2 + hop
                        reg = p2[wop][:, hop * NB : hop * NB + NB]
                        for ks in range(2):
                            lhs = t2drv[:, :, (ks * 2 + wop) * 96 : (ks * 2 + wop) * 96 + 80]
                            rhs = x2v[:, :, (hb + 2 * ks) * N + nb * NB : (hb + 2 * ks) * N + nb * NB + NB]
                            nc.tensor.matmul(
                                reg, lhs, rhs,
                                start=(ks == 0), stop=False, perf_mode=DR,
                            )
                        lhs4 = t24s[:, wop * 96 : wop * 96 + 80]
                        rhs4 = x2v[:, 0, (hb + 4) * N + nb * NB : (hb + 4) * N + nb * NB + NB]
                        nc.tensor.matmul(reg, lhs4, rhs4, start=False, stop=True)
                ew = []
                for wop in range(2):
                    e = ev.tile([96, NB], f32, tag="ea", name="e3")[0:80]
                    nc.vector.reduce_max(e, p2[wop].rearrange("q (h n) -> q n h", h=2), axis=mybir.AxisListType.X)
                    ew.append(e)
                e4 = ev.tile([96, NB], f32, tag="ec", name="e4")[0:80]
                nc.vector.tensor_max(e4, ew[0], ew[1])
                nc.scalar.sign(
                    x3v[:, ho2 % 2, (ho2 // 2) * N + nb * NB : (ho2 // 2) * N + nb * NB + NB],
                    e4, bias=b2s,
                )

            def fc_block(nb):
                p3 = ps.tile([120, NB], f32, tag="ps")
                for h5p in range(2):
                    nc.tensor.matmul(
                        p3, f1drv[:, :, h5p * 128 : h5p * 128 + 120],
                        x3v[:, :, h5p * N + nb * NB : h5p * N + nb * NB + NB],
                        start=(h5p == 0), stop=False, perf_mode=DR,
                    )
                nc.tensor.matmul(
                    p3, f14s,
                    x3v[:, 0, 2 * N + nb * NB : 2 * N + nb * NB + NB],
                    start=False, stop=True,
                )
                nc.scalar.sign(x4[:, nb * NB : nb * NB + NB], p3, bias=bf1s)

                p4 = ps.tile([84, NB], f32, tag="ps", name="p1")
                nc.tensor.matmul(p4, f2s, x4[:, nb * NB : nb * NB + NB], start=True, stop=True)
                nc.scalar.sign(x5[:, nb * NB : nb * NB + NB], p4, bias=bf2s)

                p5 = ps.tile([10, NB], f32, tag="ps")
                nc.tensor.matmul(p5, f3s, x5[:, nb * NB : nb * NB + NB], start=True, stop=True)
                nc.vector.tensor_scalar_add(outs[:, nb * NB : nb * NB + NB], p5, bf3s)
                nc.sync.dma_start(
                    out=out_d[:, nb * NB : nb * NB + NB],
                    in_=outs[:, nb * NB : nb * NB + NB],
                )

            # ---- emission: conv1, then conv2 with the fc chain emitted right
            # after conv2(4, nb) so only the last nb's fc chain trails.
            for ho2 in range(14):
                pp = []
                for nb in range(NBLK):
                    p = [ps.tile([84, 2 * NB], f32, tag="ps", name="p1") for _ in range(2)]
                    pp.append(p)
                    conv1_hi(ho2, nb, p)
                for nb in range(NBLK):
                    conv1_lo_pool(ho2, nb, pp[nb])
            for c in range(5):
                for nb in range(NBLK):
                    conv2_block(c, nb)
                    if c == 4:
                        fc_block(nb)

    nc.finalize()
    _CACHE["nc"] = nc
    return nc


def _install_ntff_hook():
    """The container's antenv stub lacks axon_hooks; synthesize it and register
    the ctypes-based NTFF profile hook from the axon boot module."""
    if "hook" in _CACHE:
        return
    _CACHE["hook"] = True
    try:
        import types
        import antenv

        if not hasattr(antenv, "axon_hooks"):
            store = {"h": None}
            m = types.ModuleType("antenv.axon_hooks")
            m.set_axon_ntff_profile_hook = lambda h: store.update(h=h)
            m.get_axon_ntff_profile_hook = lambda: store["h"]
            sys.modules["antenv.axon_hooks"] = m
            antenv.axon_hooks = m
            sys.path.insert(0, "/root/.axon_site")
            from trn_agent_boot.trn_boot import _ntff_profile_via_ctypes

            m.set_axon_ntff_profile_hook(
                _ntff_profile_via_ctypes("/opt/axon/libaxon_pjrt.so")
            )
    except Exception as e:  # profiling is best-effort
        print(f"ntff hook install failed: {e}", file=sys.stderr)


def kernel(x, w1, b1, w2, b2, wf1, bf1, wf2, bf2, wf3, bf3):
    nc = _get_nc()
    _install_ntff_hook()
    from concourse import bass_utils

    # host-side relayout: xt[core][c*32+w, h*N+n] = x[core*N+n, c, h, w]
    xr = np.ascontiguousarray(
        x.reshape(NCORES, N, 3, 32, 32).transpose(0, 2, 4, 3, 1)
    ).reshape(NCORES, 96, 32 * N)

    xh = xr.astype(np.float16)
    xl = ((xr - xh.astype(np.float32)) * 2048.0).astype(np.float16)

    t2dr, t24 = _build_t2(w2)
    f1dr, f14 = _build_f1(wf1)
    shared = {
        "t1h": _build_t1h(w1),
        "t1l": _build_t1l(w1),
        "t2dr": t2dr, "t24": t24, "f1dr": f1dr, "f14": f14,
        "f2": np.ascontiguousarray(_binarize(wf2).T).astype(BF16),
        "f3": np.ascontiguousarray(_binarize(wf3).T).astype(BF16),
        "b1v": np.tile(b1.astype(np.float32), 14).reshape(84, 1),
        "b2v": np.tile(b2.astype(np.float32), 5).reshape(80, 1),
        "bf1v": bf1.astype(np.float32).reshape(120, 1),
        "bf2v": bf2.astype(np.float32).reshape(84, 1),
        "bf3v": bf3.astype(np.float32).reshape(10, 1),
    }
    in_maps = [
        dict(shared, xh=np.ascontiguousarray(xh[i]), xl=np.ascontiguousarray(xl[i]))
        for i in range(NCORES)
    ]

    res = bass_utils.run_bass_kernel_spmd(
        nc, in_maps, core_ids=list(range(NCORES)),
        trace=bool(int(os.environ.get("KERNEL_TRACE", "0"))),
    )
    if res.exec_time_ns is not None:
        print(f"HW exec time: {res.exec_time_ns} ns")
    out = np.stack([r["out"] for r in res.results])  # [8, 10, N]
    return np.ascontiguousarray(out.transpose(0, 2, 1)).reshape(B, 10).astype(np.float32)


# revision 13
# speedup vs baseline: 1.0508x; 1.0508x over previous
"""BinaryLeNet5 forward on 8 TRN2 NeuronCores, pure data parallel (1024 imgs/core).

Mapping summary (per core):
  conv1: kh-accumulated banded-Toeplitz matmuls, split into a 2-level precision
         ladder that is ~f32-exact but runs the PE at 2 cycles/row (vs 4 for
         native f32 matmuls):
           hi pass : fp16(x) against fp16 Toeplitz (+-1 exact), 1 cyc/row
           lo pass : fp16((x-hi)*2^11) against the Toeplitz scaled +-2^-11
                     (exact fp16 normals), 1 cyc/row
         All products are exact; PSUM accumulates in f32 -> ~22 effective
         mantissa bits on x, which simulation shows gives 0 mismatches.
  pool+sign: maxpool pairs are (a) psum free-dim pairs (ho parity) and (b) two
         PSUM tiles (wo parity) -> dense DVE maxes, then ACT Sign with f32
         per-partition bias. Sign output written twice into an fp8 pair layout
         x2dr[p, s, h] (slot s holds row h+s) so conv2 can contract kh-pairs.
  conv2: inputs/weights exactly +-1 in e4m3 -> kh taps {0,1},{2,3} are two
         DoubleRow matmuls (pair axis = kh tap), tap 4 a plain fp8 matmul.
  fc1:   same DoubleRow kh-pairing over the 5 h-blocks of the 400-dim input.
  fc2/fc3: tiny, bf16 (+-1 exact), f32 PSUM, biases in f32 via ACT bias.
  hardtanh drops out everywhere: sign(clip(x)) == sign(x), max(clip) == clip(max).

DMA: conv1 weights first, then the x streams split across both HWDGE rings
(sync: fp16 hi stream, scalar: fp16 scaled-residual stream), then the
conv2/fc weights, so compute starts ~14us in.

Output written as [10, 1024] per core, transposed/stacked on host.
"""

import os
import sys

import numpy as np

sys.path.insert(0, "/opt/trn_rl_repo")

import ml_dtypes  # noqa: E402

BF16 = ml_dtypes.bfloat16
F8E4 = ml_dtypes.float8_e4m3
F8E5 = ml_dtypes.float8_e5m2

B = 8192
NCORES = 8
N = B // NCORES  # 1024 images per core
NBLK = 2  # n blocks of 512 columns
NB = N // NBLK  # 512
HCH = 2  # h rows per x sbuf tile


def _binarize(w):
    return np.where(w >= 0, 1.0, -1.0).astype(np.float32)


def _build_t1h(w1):
    # t1[c*32+wi, kh*168 + par*84 + wo2*6 + o] = w1b[o,c,kh,kw]
    #   wo = 2*wo2 + par (par = wo parity), kw = wi - wo, valid 0<=kw<5
    w1b = _binarize(w1)  # [6,3,5,5]
    t1 = np.zeros((96, 5 * 168), np.float32)
    for kh in range(5):
        for par in range(2):
            for wo2 in range(14):
                wo = 2 * wo2 + par
                for o in range(6):
                    col = kh * 168 + par * 84 + wo2 * 6 + o
                    for c in range(3):
                        for kw in range(5):
                            wi = wo + kw
                            if wi < 32:
                                t1[c * 32 + wi, col] = w1b[o, c, kh, kw]
    return t1.astype(np.float16)


def _build_t1l(w1):
    # lo-pass weights: the fp16 Toeplitz scaled by 2^-11 (exact fp16 normals).
    return (_build_t1h(w1).astype(np.float32) * 2.0**-11).astype(np.float16)


def _build_t2(w2):
    # DR pairs: t2dr[w2*6+c, s, (ks*2+wop)*96 + wo2*16+o] = w2b[o,c,2ks+s,kw]
    # tap4:     t24 [w2*6+c, wop*96 + wo2*16+o] = w2b[o,c,4,kw]
    w2b = _binarize(w2)  # [16,6,5,5]
    t2dr = np.zeros((84, 2, 4 * 96), np.float32)
    t24 = np.zeros((84, 2 * 96), np.float32)
    for wop in range(2):
        for wo2 in range(5):
            wo = 2 * wo2 + wop
            for o in range(16):
                for c in range(6):
                    for kw in range(5):
                        w2i = wo + kw
                        if w2i >= 14:
                            continue
                        row = w2i * 6 + c
                        for ks in range(2):
                            for s in range(2):
                                t2dr[row, s, (ks * 2 + wop) * 96 + wo2 * 16 + o] = w2b[
                                    o, c, 2 * ks + s, kw
                                ]
                        t24[row, wop * 96 + wo2 * 16 + o] = w2b[o, c, 4, kw]
    return (
        np.ascontiguousarray(t2dr.reshape(84, 2 * 384)).astype(F8E4),
        t24.astype(F8E4),
    )


def _build_f1(wf1):
    # DR pairs: f1dr[w*16+o, s, h5p*128 + f] = wf1b[f, o*25+(2*h5p+s)*5+w]
    # tap4:     f14 [w*16+o, f] = wf1b[f, o*25+20+w]
    wf1b = _binarize(wf1)  # [120, 400]
    f1dr = np.zeros((80, 2, 2 * 128), np.float32)
    f14 = np.zeros((80, 120), np.float32)
    for w in range(5):
        for o in range(16):
            row = w * 16 + o
            for h5p in range(2):
                for s in range(2):
                    f1dr[row, s, h5p * 128 : h5p * 128 + 120] = wf1b[
                        :, o * 25 + (2 * h5p + s) * 5 + w
                    ]
            f14[row, :] = wf1b[:, o * 25 + 20 + w]
    return (
        np.ascontiguousarray(f1dr.reshape(80, 2 * 256)).astype(F8E4),
        f14.astype(F8E4),
    )


_CACHE = {}


def _get_nc():
    if "nc" in _CACHE:
        return _CACHE["nc"]
    import concourse.bacc as bacc
    import concourse.mybir as mybir
    import concourse.tile as tile

    f32 = mybir.dt.float32
    f16 = mybir.dt.float16
    bf16 = mybir.dt.bfloat16
    f8e4 = mybir.dt.float8e4
    f8e5 = mybir.dt.float8e5
    DR = mybir.MatmulPerfMode.DoubleRow

    nc = bacc.Bacc()
    xh_d = nc.dram_tensor("xh", [96, 32 * N], f16, kind="ExternalInput")
    xl_d = nc.dram_tensor("xl", [96, 32 * N], f16, kind="ExternalInput")
    t1h_d = nc.dram_tensor("t1h", [96, 840], f16, kind="ExternalInput")
    t1l_d = nc.dram_tensor("t1l", [96, 840], f16, kind="ExternalInput")
    t2dr_d = nc.dram_tensor("t2dr", [84, 768], f8e4, kind="ExternalInput")
    t24_d = nc.dram_tensor("t24", [84, 192], f8e4, kind="ExternalInput")
    f1dr_d = nc.dram_tensor("f1dr", [80, 512], f8e4, kind="ExternalInput")
    f14_d = nc.dram_tensor("f14", [80, 120], f8e4, kind="ExternalInput")
    f2_d = nc.dram_tensor("f2", [120, 84], bf16, kind="ExternalInput")
    f3_d = nc.dram_tensor("f3", [84, 10], bf16, kind="ExternalInput")
    b1_d = nc.dram_tensor("b1v", [84, 1], f32, kind="ExternalInput")
    b2_d = nc.dram_tensor("b2v", [80, 1], f32, kind="ExternalInput")
    bf1_d = nc.dram_tensor("bf1v", [120, 1], f32, kind="ExternalInput")
    bf2_d = nc.dram_tensor("bf2v", [84, 1], f32, kind="ExternalInput")
    bf3_d = nc.dram_tensor("bf3v", [10, 1], f32, kind="ExternalInput")
    out_d = nc.dram_tensor("out", [10, N], f32, kind="ExternalOutput")

    with tile.TileContext(nc) as tc:
        with (
            tc.tile_pool(name="xtp", bufs=1) as xtp,
            tc.tile_pool(name="wts", bufs=1) as wts,
            tc.tile_pool(name="acts", bufs=1) as acts,
            tc.tile_pool(name="ev", bufs=3) as ev,
            tc.tile_pool(name="ps", bufs=4, space="PSUM") as ps,
        ):
            # ---- DMA issue order: conv1 weights, then the x streams
            # interleaved across BOTH HWDGE rings (hi/lo tiles alternate
            # rings) so the early rows of both streams arrive in parallel,
            # then the weights that are only needed from conv2 onward.
            t1hs = wts.tile([96, 840], f16, tag="t1h")
            nc.sync.dma_start(out=t1hs, in_=t1h_d[:, :])
            t1ls = wts.tile([96, 840], f16, tag="t1l")
            nc.scalar.dma_start(out=t1ls, in_=t1l_d[:, :])
            b1s = wts.tile([84, 1], f32, tag="b1")
            nc.scalar.dma_start(out=b1s, in_=b1_d[:, :])

            xhs, xls = [], []
            for k in range(32 // HCH):
                sl = slice(k * HCH * N, (k + 1) * HCH * N)
                ring_h = nc.sync if k % 2 == 0 else nc.scalar
                ring_l = nc.scalar if k % 2 == 0 else nc.sync
                th = xtp.tile([96, HCH * N], f16, tag=f"xh{k}", name=f"xh{k}")
                ring_h.dma_start(out=th, in_=xh_d[:, sl])
                xhs.append(th)
                tl = xtp.tile([96, HCH * N], f16, tag=f"xl{k}", name=f"xl{k}")
                ring_l.dma_start(out=tl, in_=xl_d[:, sl])
                xls.append(tl)

            t2drs = wts.tile([84, 768], f8e4, tag="t2dr")
            nc.scalar.dma_start(out=t2drs, in_=t2dr_d[:, :])
            t24s = wts.tile([84, 192], f8e4, tag="t24")
            nc.scalar.dma_start(out=t24s, in_=t24_d[:, :])
            f1drs = wts.tile([80, 512], f8e4, tag="f1dr")
            nc.scalar.dma_start(out=f1drs, in_=f1dr_d[:, :])
            f14s = wts.tile([80, 120], f8e4, tag="f14")
            nc.scalar.dma_start(out=f14s, in_=f14_d[:, :])
            f2s = wts.tile([120, 84], bf16, tag="f2")
            nc.scalar.dma_start(out=f2s, in_=f2_d[:, :])
            f3s = wts.tile([84, 10], bf16, tag="f3")
            nc.scalar.dma_start(out=f3s, in_=f3_d[:, :])
            b2s = wts.tile([80, 1], f32, tag="b2")
            nc.scalar.dma_start(out=b2s, in_=b2_d[:, :])
            bf1s = wts.tile([120, 1], f32, tag="bf1")
            nc.scalar.dma_start(out=bf1s, in_=bf1_d[:, :])
            bf2s = wts.tile([84, 1], f32, tag="bf2")
            nc.scalar.dma_start(out=bf2s, in_=bf2_d[:, :])
            bf3s = wts.tile([10, 1], f32, tag="bf3")
            nc.scalar.dma_start(out=bf3s, in_=bf3_d[:, :])

            def xh_row(h, nb):  # [96, NB] fp16 slice for input row h
                return xhs[h // HCH][
                    :, (h % HCH) * N + nb * NB : (h % HCH) * N + nb * NB + NB
                ]

            def xl_row(h, nb):  # [96, NB] fp16 scaled-residual slice
                return xls[h // HCH][
                    :, (h % HCH) * N + nb * NB : (h % HCH) * N + nb * NB + NB
                ]

            # One consumer-engine 'touch' per DMA'd bias tile: the touch op
            # carries the DMA wait, so later ops on that engine need no extra
            # wait slot (TRN2 engine instructions have a single wait slot).
            tb1 = wts.tile([84, 1], f32, tag="tb1")
            nc.scalar.copy(tb1, b1s)
            tb2 = wts.tile([80, 1], f32, tag="tb2")
            nc.scalar.copy(tb2, b2s)
            tb3 = wts.tile([120, 1], f32, tag="tb3")
            nc.scalar.copy(tb3, bf1s)
            tb4 = wts.tile([84, 1], f32, tag="tb4")
            nc.scalar.copy(tb4, bf2s)
            tb5 = wts.tile([10, 1], f32, tag="tb5")
            nc.vector.tensor_copy(tb5, bf3s)

            x2dr = acts.tile([84, 2 * 14 * N], f8e4, tag="x2dr")
            x3dr = acts.tile([80, 2 * 3 * N], f8e4, tag="x3dr")
            x4 = acts.tile([120, N], bf16, tag="x4")
            x5 = acts.tile([84, N], bf16, tag="x5")
            outs = acts.tile([10, N], f32, tag="outs")
            x2v = x2dr.rearrange("p (s f) -> p s f", s=2)
            x3v = x3dr.rearrange("p (s f) -> p s f", s=2)
            t2drv = t2drs.rearrange("p (s f) -> p s f", s=2)
            f1drv = f1drs.rearrange("p (s f) -> p s f", s=2)

            # ---- block emitters ----
            # psum tile [84, 1024] = (hop 2) x (n 512) blocks; ho-pair pooled
            # in free dim by reduce_max; wo-pair = tensor_max of the two parity
            # chunks (same partitions). hi fp16 pass then fp16 scaled-residual
            # pass accumulate into the same PSUM region. hi and lo are emitted
            # separately so the start of conv1 can run hi-only while the lo
            # stream is still arriving.
            def conv1_hi(ho2, nb, p):
                for par in range(2):
                    for kh in range(5):
                        lhs = t1hs[:, kh * 168 + par * 84 : kh * 168 + par * 84 + 84]
                        for hop in range(2):
                            nc.tensor.matmul(
                                p[par][:, hop * NB : hop * NB + NB], lhs,
                                xh_row(2 * ho2 + hop + kh, nb),
                                start=(kh == 0), stop=False,
                            )

            def conv1_lo_pool(ho2, nb, p):
                for par in range(2):
                    for kh in range(5):
                        lhsl = t1ls[:, kh * 168 + par * 84 : kh * 168 + par * 84 + 84]
                        for hop in range(2):
                            nc.tensor.matmul(
                                p[par][:, hop * NB : hop * NB + NB], lhsl,
                                xl_row(2 * ho2 + hop + kh, nb),
                                start=False, stop=(kh == 4),
                            )
                e1 = []
                for par in range(2):
                    e = ev.tile([96, NB], f32, tag="ea", name="e1")[0:84]
                    nc.vector.reduce_max(e, p[par].rearrange("q (h n) -> q n h", h=2), axis=mybir.AxisListType.X)
                    e1.append(e)
                e2 = ev.tile([96, NB], f32, tag="ec", name="e2")[0:84]
                nc.vector.tensor_max(e2, e1[0], e1[1])
                nc.scalar.sign(
                    x2v[:, 0, ho2 * N + nb * NB : ho2 * N + nb * NB + NB],
                    e2, bias=b1s,
                )
                # duplicate into slot 1 at row ho2-1; conv2 reads slot 1 only
                # up to row 11, so the ho2-1 >= 12 copies are skipped.
                if 0 < ho2 <= 12:
                    nc.scalar.sign(
                        x2v[:, 1, (ho2 - 1) * N + nb * NB : (ho2 - 1) * N + nb * NB + NB],
                        e2, bias=b1s,
                    )

            def conv2_block(ho2, nb):
                # kh taps {0,1} and {2,3} via DoubleRow pair slots, tap 4 plain.
                p2 = [ps.tile([80, 2 * NB], f32, tag="ps", name="p2") for _ in range(2)]
                for wop in range(2):
                    for hop in range(2):
                        hb = 2 * ho2 + hop
                        reg = p2[wop][:, hop * NB : hop * NB + NB]
                        for ks in range(2):
                            lhs = t2drv[:, :, (ks * 2 + wop) * 96 : (ks * 2 + wop) * 96 + 80]
                            rhs = x2v[:, :, (hb + 2 * ks) * N + nb * NB : (hb + 2 * ks) * N + nb * NB + NB]
                            nc.tensor.matmul(
                                reg, lhs, rhs,
                                start=(ks == 0), stop=False, perf_mode=DR,
                            )
                        lhs4 = t24s[:, wop * 96 : wop * 96 + 80]
                        rhs4 = x2v[:, 0, (hb + 4) * N + nb * NB : (hb + 4) * N + nb * NB + NB]
                        nc.tensor.matmul(reg, lhs4, rhs4, start=False, stop=True)
                ew = []
                for wop in range(2):
                    e = ev.tile([96, NB], f32, tag="ea", name="e3")[0:80]
                    nc.vector.reduce_max(e, p2[wop].rearrange("q (h n) -> q n h", h=2), axis=mybir.AxisListType.X)
                    ew.append(e)
                e4 = ev.tile([96, NB], f32, tag="ec", name="e4")[0:80]
                nc.vector.tensor_max(e4, ew[0], ew[1])
                nc.scalar.sign(
                    x3v[:, ho2 % 2, (ho2 // 2) * N + nb * NB : (ho2 // 2) * N + nb * NB + NB],
                    e4, bias=b2s,
                )

            def fc_block(nb):
                p3 = ps.tile([120, NB], f32, tag="ps")
                for h5p in range(2):
                    nc.tensor.matmul(
                        p3, f1drv[:, :, h5p * 128 : h5p * 128 + 120],
                        x3v[:, :, h5p * N + nb * NB : h5p * N + nb * NB + NB],
                        start=(h5p == 0), stop=False, perf_mode=DR,
                    )
                nc.tensor.matmul(
                    p3, f14s,
                    x3v[:, 0, 2 * N + nb * NB : 2 * N + nb * NB + NB],
                    start=False, stop=True,
                )
                nc.scalar.sign(x4[:, nb * NB : nb * NB + NB], p3, bias=bf1s)

                p4 = ps.tile([84, NB], f32, tag="ps", name="p1")
                nc.tensor.matmul(p4, f2s, x4[:, nb * NB : nb * NB + NB], start=True, stop=True)
                nc.scalar.sign(x5[:, nb * NB : nb * NB + NB], p4, bias=bf2s)

                p5 = ps.tile([10, NB], f32, tag="ps")
                nc.tensor.matmul(p5, f3s, x5[:, nb * NB : nb * NB + NB], start=True, stop=True)
                nc.vector.tensor_scalar_add(outs[:, nb * NB : nb * NB + NB], p5, bf3s)
                nc.sync.dma_start(
                    out=out_d[:, nb * NB : nb * NB + NB],
                    in_=outs[:, nb * NB : nb * NB + NB],
                )

            # ---- emission: conv1, then conv2 with the fc chain emitted right
            # after conv2(4, nb) so only the last nb's fc chain trails.
            for ho2 in range(14):
                for nb in range(NBLK):
                    p = [ps.tile([84, 2 * NB], f32, tag="ps", name="p1") for _ in range(2)]
                    conv1_hi(ho2, nb, p)
                    conv1_lo_pool(ho2, nb, p)
            for c in range(5):
                for nb in range(NBLK):
                    conv2_block(c, nb)
                    if c == 4:
                        fc_block(nb)

    nc.finalize()
    _CACHE["nc"] = nc
    return nc


def _install_ntff_hook():
    """The container's antenv stub lacks axon_hooks; synthesize it and register
    the ctypes-based NTFF profile hook from the axon boot module."""
    if "hook" in _CACHE:
        return
    _CACHE["hook"] = True
    try:
        import types
        import antenv

        if not hasattr(antenv, "axon_hooks"):
            store = {"h": None}
            m = types.ModuleType("antenv.axon_hooks")
            m.set_axon_ntff_profile_hook = lambda h: store.update(h=h)
            m.get_axon_ntff_profile_hook = lambda: store["h"]
            sys.modules["antenv.axon_hooks"] = m
            antenv.axon_hooks = m
            sys.path.insert(0, "/root/.axon_site")
            from trn_agent_boot.trn_boot import _ntff_profile_via_ctypes

            m.set_axon_ntff_profile_hook(
                _ntff_profile_via_ctypes("/opt/axon/libaxon_pjrt.so")
            )
    except Exception as e:  # profiling is best-effort
        print(f"ntff hook install failed: {e}", file=sys.stderr)


def kernel(x, w1, b1, w2, b2, wf1, bf1, wf2, bf2, wf3, bf3):
    nc = _get_nc()
    _install_ntff_hook()
    from concourse import bass_utils

    # host-side relayout: xt[core][c*32+w, h*N+n] = x[core*N+n, c, h, w]
    xr = np.ascontiguousarray(
        x.reshape(NCORES, N, 3, 32, 32).transpose(0, 2, 4, 3, 1)
    ).reshape(NCORES, 96, 32 * N)

    xh = xr.astype(np.float16)
    xl = ((xr - xh.astype(np.float32)) * 2048.0).astype(np.float16)

    t2dr, t24 = _build_t2(w2)
    f1dr, f14 = _build_f1(wf1)
    shared = {
        "t1h": _build_t1h(w1),
        "t1l": _build_t1l(w1),
        "t2dr": t2dr, "t24": t24, "f1dr": f1dr, "f14": f14,
        "f2": np.ascontiguousarray(_binarize(wf2).T).astype(BF16),
        "f3": np.ascontiguousarray(_binarize(wf3).T).astype(BF16),
        "b1v": np.tile(b1.astype(np.float32), 14).reshape(84, 1),
        "b2v": np.tile(b2.astype(np.float32), 5).reshape(80, 1),
        "bf1v": bf1.astype(np.float32).reshape(120, 1),
        "bf2v": bf2.astype(np.float32).reshape(84, 1),
        "bf3v": bf3.astype(np.float32).reshape(10, 1),
    }
    in_maps = [
        dict(shared, xh=np.ascontiguousarray(xh[i]), xl=np.ascontiguousarray(xl[i]))
        for i in range(NCORES)
    ]

    res = bass_utils.run_bass_kernel_spmd(
        nc, in_maps, core_ids=list(range(NCORES)),
        trace=bool(int(os.environ.get("KERNEL_TRACE", "0"))),
    )
    if res.exec_time_ns is not None:
        print(f"HW exec time: {res.exec_time_ns} ns")
    out = np.stack([r["out"] for r in res.results])  # [8, 10, N]
    return np.ascontiguousarray(out.transpose(0, 2, 1)).reshape(B, 10).astype(np.float32)


# revision 14
# speedup vs baseline: 1.0546x; 1.0037x over previous
"""BinaryLeNet5 forward on 8 TRN2 NeuronCores, pure data parallel (1024 imgs/core).

Mapping summary (per core):
  conv1: kh-accumulated banded-Toeplitz matmuls, split into a 2-level precision
         ladder that is ~f32-exact but runs the PE at 2 cycles/row (vs 4 for
         native f32 matmuls):
           hi pass : fp16(x) against fp16 Toeplitz (+-1 exact), 1 cyc/row
           lo pass : fp16((x-hi)*2^11) against the Toeplitz scaled +-2^-11
                     (exact fp16 normals), 1 cyc/row
         All products are exact; PSUM accumulates in f32 -> ~22 effective
         mantissa bits on x, which simulation shows gives 0 mismatches.
  pool+sign: maxpool pairs are (a) psum free-dim pairs (ho parity) and (b) two
         PSUM tiles (wo parity) -> dense DVE maxes, then ACT Sign with f32
         per-partition bias. Sign output written twice into an fp8 pair layout
         x2dr[p, s, h] (slot s holds row h+s) so conv2 can contract kh-pairs.
  conv2: inputs/weights exactly +-1 in e4m3 -> kh taps {0,1},{2,3} are two
         DoubleRow matmuls (pair axis = kh tap), tap 4 a plain fp8 matmul.
  fc1:   same DoubleRow kh-pairing over the 5 h-blocks of the 400-dim input.
  fc2/fc3: tiny, bf16 (+-1 exact), f32 PSUM, biases in f32 via ACT bias.
  hardtanh drops out everywhere: sign(clip(x)) == sign(x), max(clip) == clip(max).

DMA: conv1 weights first, then the x streams split across both HWDGE rings
(sync: fp16 hi stream, scalar: fp16 scaled-residual stream), then the
conv2/fc weights, so compute starts ~14us in.

Output written as [10, 1024] per core, transposed/stacked on host.
"""

import os
import sys

import numpy as np

sys.path.insert(0, "/opt/trn_rl_repo")

import ml_dtypes  # noqa: E402

BF16 = ml_dtypes.bfloat16
F8E4 = ml_dtypes.float8_e4m3
F8E5 = ml_dtypes.float8_e5m2

B = 8192
NCORES = 8
N = B // NCORES  # 1024 images per core
NBLK = 2  # n blocks of 512 columns
NB = N // NBLK  # 512
HCH = 2  # h rows per x sbuf tile


def _binarize(w):
    return np.where(w >= 0, 1.0, -1.0).astype(np.float32)


def _build_t1h(w1):
    # t1[c*32+wi, kh*168 + par*84 + wo2*6 + o] = w1b[o,c,kh,kw]
    #   wo = 2*wo2 + par (par = wo parity), kw = wi - wo, valid 0<=kw<5
    w1b = _binarize(w1)  # [6,3,5,5]
    t1 = np.zeros((96, 5 * 168), np.float32)
    for kh in range(5):
        for par in range(2):
            for wo2 in range(14):
                wo = 2 * wo2 + par
                for o in range(6):
                    col = kh * 168 + par * 84 + wo2 * 6 + o
                    for c in range(3):
                        for kw in range(5):
                            wi = wo + kw
                            if wi < 32:
                                t1[c * 32 + wi, col] = w1b[o, c, kh, kw]
    return t1.astype(np.float16)


def _build_t1l(w1):
    # lo-pass weights: the fp16 Toeplitz scaled by 2^-11 (exact fp16 normals).
    return (_build_t1h(w1).astype(np.float32) * 2.0**-11).astype(np.float16)


def _build_t2(w2):
    # DR pairs: t2dr[w2*6+c, s, (ks*2+wop)*96 + wo2*16+o] = w2b[o,c,2ks+s,kw]
    # tap4:     t24 [w2*6+c, wop*96 + wo2*16+o] = w2b[o,c,4,kw]
    w2b = _binarize(w2)  # [16,6,5,5]
    t2dr = np.zeros((84, 2, 4 * 96), np.float32)
    t24 = np.zeros((84, 2 * 96), np.float32)
    for wop in range(2):
        for wo2 in range(5):
            wo = 2 * wo2 + wop
            for o in range(16):
                for c in range(6):
                    for kw in range(5):
                        w2i = wo + kw
                        if w2i >= 14:
                            continue
                        row = w2i * 6 + c
                        for ks in range(2):
                            for s in range(2):
                                t2dr[row, s, (ks * 2 + wop) * 96 + wo2 * 16 + o] = w2b[
                                    o, c, 2 * ks + s, kw
                                ]
                        t24[row, wop * 96 + wo2 * 16 + o] = w2b[o, c, 4, kw]
    return (
        np.ascontiguousarray(t2dr.reshape(84, 2 * 384)).astype(F8E4),
        t24.astype(F8E4),
    )


def _build_f1(wf1):
    # DR pairs: f1dr[w*16+o, s, h5p*128 + f] = wf1b[f, o*25+(2*h5p+s)*5+w]
    # tap4:     f14 [w*16+o, f] = wf1b[f, o*25+20+w]
    wf1b = _binarize(wf1)  # [120, 400]
    f1dr = np.zeros((80, 2, 2 * 128), np.float32)
    f14 = np.zeros((80, 120), np.float32)
    for w in range(5):
        for o in range(16):
            row = w * 16 + o
            for h5p in range(2):
                for s in range(2):
                    f1dr[row, s, h5p * 128 : h5p * 128 + 120] = wf1b[
                        :, o * 25 + (2 * h5p + s) * 5 + w
                    ]
            f14[row, :] = wf1b[:, o * 25 + 20 + w]
    return (
        np.ascontiguousarray(f1dr.reshape(80, 2 * 256)).astype(F8E4),
        f14.astype(F8E4),
    )


_CACHE = {}


def _get_nc():
    if "nc" in _CACHE:
        return _CACHE["nc"]
    import concourse.bacc as bacc
    import concourse.mybir as mybir
    import concourse.tile as tile

    f32 = mybir.dt.float32
    f16 = mybir.dt.float16
    bf16 = mybir.dt.bfloat16
    f8e4 = mybir.dt.float8e4
    f8e5 = mybir.dt.float8e5
    DR = mybir.MatmulPerfMode.DoubleRow

    nc = bacc.Bacc()
    xh_d = nc.dram_tensor("xh", [96, 32 * N], f16, kind="ExternalInput")
    xl_d = nc.dram_tensor("xl", [96, 32 * N], f16, kind="ExternalInput")
    t1h_d = nc.dram_tensor("t1h", [96, 840], f16, kind="ExternalInput")
    t1l_d = nc.dram_tensor("t1l", [96, 840], f16, kind="ExternalInput")
    t2dr_d = nc.dram_tensor("t2dr", [84, 768], f8e4, kind="ExternalInput")
    t24_d = nc.dram_tensor("t24", [84, 192], f8e4, kind="ExternalInput")
    f1dr_d = nc.dram_tensor("f1dr", [80, 512], f8e4, kind="ExternalInput")
    f14_d = nc.dram_tensor("f14", [80, 120], f8e4, kind="ExternalInput")
    f2_d = nc.dram_tensor("f2", [120, 84], bf16, kind="ExternalInput")
    f3_d = nc.dram_tensor("f3", [84, 10], bf16, kind="ExternalInput")
    b1_d = nc.dram_tensor("b1v", [84, 1], f32, kind="ExternalInput")
    b2_d = nc.dram_tensor("b2v", [80, 1], f32, kind="ExternalInput")
    bf1_d = nc.dram_tensor("bf1v", [120, 1], f32, kind="ExternalInput")
    bf2_d = nc.dram_tensor("bf2v", [84, 1], f32, kind="ExternalInput")
    bf3_d = nc.dram_tensor("bf3v", [10, 1], f32, kind="ExternalInput")
    out_d = nc.dram_tensor("out", [10, N], f32, kind="ExternalOutput")

    with tile.TileContext(nc) as tc:
        with (
            tc.tile_pool(name="xtp", bufs=1) as xtp,
            tc.tile_pool(name="wts", bufs=1) as wts,
            tc.tile_pool(name="acts", bufs=1) as acts,
            tc.tile_pool(name="ev", bufs=3) as ev,
            tc.tile_pool(name="ps", bufs=4, space="PSUM") as ps,
        ):
            # ---- DMA issue order: conv1 weights, then the x streams
            # interleaved across BOTH HWDGE rings (hi/lo tiles alternate
            # rings) so the early rows of both streams arrive in parallel,
            # then the weights that are only needed from conv2 onward.
            t1hs = wts.tile([96, 840], f16, tag="t1h")
            nc.sync.dma_start(out=t1hs, in_=t1h_d[:, :])
            t1ls = wts.tile([96, 840], f16, tag="t1l")
            nc.scalar.dma_start(out=t1ls, in_=t1l_d[:, :])
            b1s = wts.tile([84, 1], f32, tag="b1")
            nc.scalar.dma_start(out=b1s, in_=b1_d[:, :])

            xhs, xls = [], []
            for k in range(32 // HCH):
                sl = slice(k * HCH * N, (k + 1) * HCH * N)
                ring_h = nc.sync if k % 2 == 0 else nc.scalar
                ring_l = nc.scalar if k % 2 == 0 else nc.sync
                th = xtp.tile([96, HCH * N], f16, tag=f"xh{k}", name=f"xh{k}")
                ring_h.dma_start(out=th, in_=xh_d[:, sl])
                xhs.append(th)
                tl = xtp.tile([96, HCH * N], f16, tag=f"xl{k}", name=f"xl{k}")
                ring_l.dma_start(out=tl, in_=xl_d[:, sl])
                xls.append(tl)

            t2drs = wts.tile([84, 768], f8e4, tag="t2dr")
            nc.scalar.dma_start(out=t2drs, in_=t2dr_d[:, :])
            t24s = wts.tile([84, 192], f8e4, tag="t24")
            nc.scalar.dma_start(out=t24s, in_=t24_d[:, :])
            f1drs = wts.tile([80, 512], f8e4, tag="f1dr")
            nc.scalar.dma_start(out=f1drs, in_=f1dr_d[:, :])
            f14s = wts.tile([80, 120], f8e4, tag="f14")
            nc.scalar.dma_start(out=f14s, in_=f14_d[:, :])
            f2s = wts.tile([120, 84], bf16, tag="f2")
            nc.scalar.dma_start(out=f2s, in_=f2_d[:, :])
            f3s = wts.tile([84, 10], bf16, tag="f3")
            nc.scalar.dma_start(out=f3s, in_=f3_d[:, :])
            b2s = wts.tile([80, 1], f32, tag="b2")
            nc.scalar.dma_start(out=b2s, in_=b2_d[:, :])
            bf1s = wts.tile([120, 1], f32, tag="bf1")
            nc.scalar.dma_start(out=bf1s, in_=bf1_d[:, :])
            bf2s = wts.tile([84, 1], f32, tag="bf2")
            nc.scalar.dma_start(out=bf2s, in_=bf2_d[:, :])
            bf3s = wts.tile([10, 1], f32, tag="bf3")
            nc.scalar.dma_start(out=bf3s, in_=bf3_d[:, :])

            def xh_row(h, nb):  # [96, NB] fp16 slice for input row h
                return xhs[h // HCH][
                    :, (h % HCH) * N + nb * NB : (h % HCH) * N + nb * NB + NB
                ]

            def xl_row(h, nb):  # [96, NB] fp16 scaled-residual slice
                return xls[h // HCH][
                    :, (h % HCH) * N + nb * NB : (h % HCH) * N + nb * NB + NB
                ]

            # One consumer-engine 'touch' per DMA'd bias tile: the touch op
            # carries the DMA wait, so later ops on that engine need no extra
            # wait slot (TRN2 engine instructions have a single wait slot).
            tb1 = wts.tile([84, 1], f32, tag="tb1")
            nc.scalar.copy(tb1, b1s)
            tb2 = wts.tile([80, 1], f32, tag="tb2")
            nc.scalar.copy(tb2, b2s)
            tb3 = wts.tile([120, 1], f32, tag="tb3")
            nc.scalar.copy(tb3, bf1s)
            tb4 = wts.tile([84, 1], f32, tag="tb4")
            nc.scalar.copy(tb4, bf2s)
            tb5 = wts.tile([10, 1], f32, tag="tb5")
            nc.vector.tensor_copy(tb5, bf3s)

            x2dr = acts.tile([84, 2 * 14 * N], f8e4, tag="x2dr")
            x3dr = acts.tile([80, 2 * 3 * N], f8e4, tag="x3dr")
            x4 = acts.tile([120, N], bf16, tag="x4")
            x5 = acts.tile([84, N], bf16, tag="x5")
            outs = acts.tile([10, N], f32, tag="outs")
            x2v = x2dr.rearrange("p (s f) -> p s f", s=2)
            x3v = x3dr.rearrange("p (s f) -> p s f", s=2)
            t2drv = t2drs.rearrange("p (s f) -> p s f", s=2)
            f1drv = f1drs.rearrange("p (s f) -> p s f", s=2)

            # ---- block emitters ----
            # psum tile [84, 1024] = (hop 2) x (n 512) blocks; ho-pair pooled
            # in free dim by reduce_max; wo-pair = tensor_max of the two parity
            # chunks (same partitions). hi fp16 pass then fp16 scaled-residual
            # pass accumulate into the same PSUM region. hi and lo are emitted
            # separately so the start of conv1 can run hi-only while the lo
            # stream is still arriving.
            def conv1_hi(ho2, nb, p):
                for par in range(2):
                    for kh in range(5):
                        lhs = t1hs[:, kh * 168 + par * 84 : kh * 168 + par * 84 + 84]
                        for hop in range(2):
                            nc.tensor.matmul(
                                p[par][:, hop * NB : hop * NB + NB], lhs,
                                xh_row(2 * ho2 + hop + kh, nb),
                                start=(kh == 0), stop=False,
                            )

            def conv1_lo_pool(ho2, nb, p):
                for par in range(2):
                    for kh in range(5):
                        lhsl = t1ls[:, kh * 168 + par * 84 : kh * 168 + par * 84 + 84]
                        for hop in range(2):
                            nc.tensor.matmul(
                                p[par][:, hop * NB : hop * NB + NB], lhsl,
                                xl_row(2 * ho2 + hop + kh, nb),
                                start=False, stop=(kh == 4),
                            )
                e1 = []
                for par in range(2):
                    e = ev.tile([96, NB], f32, tag="ea", name="e1")[0:84]
                    nc.vector.reduce_max(e, p[par].rearrange("q (h n) -> q n h", h=2), axis=mybir.AxisListType.X)
                    e1.append(e)
                e2 = ev.tile([96, NB], f32, tag="ec", name="e2")[0:84]
                nc.vector.tensor_max(e2, e1[0], e1[1])
                nc.scalar.sign(
                    x2v[:, 0, ho2 * N + nb * NB : ho2 * N + nb * NB + NB],
                    e2, bias=b1s,
                )
                # duplicate into slot 1 at row ho2-1; conv2 reads slot 1 only
                # up to row 11, so the ho2-1 >= 12 copies are skipped.
                if 0 < ho2 <= 12:
                    nc.scalar.sign(
                        x2v[:, 1, (ho2 - 1) * N + nb * NB : (ho2 - 1) * N + nb * NB + NB],
                        e2, bias=b1s,
                    )

            def conv2_block(ho2, nb):
                # kh taps {0,1} and {2,3} via DoubleRow pair slots, tap 4 plain.
                p2 = [ps.tile([80, 2 * NB], f32, tag="ps", name="p2") for _ in range(2)]
                for wop in range(2):
                    for hop in range(2):
                        hb = 2 * ho2 + hop
                        reg = p2[wop][:, hop * NB : hop * NB + NB]
                        for ks in range(2):
                            lhs = t2drv[:, :, (ks * 2 + wop) * 96 : (ks * 2 + wop) * 96 + 80]
                            rhs = x2v[:, :, (hb + 2 * ks) * N + nb * NB : (hb + 2 * ks) * N + nb * NB + NB]
                            nc.tensor.matmul(
                                reg, lhs, rhs,
                                start=(ks == 0), stop=False, perf_mode=DR,
                            )
                        lhs4 = t24s[:, wop * 96 : wop * 96 + 80]
                        rhs4 = x2v[:, 0, (hb + 4) * N + nb * NB : (hb + 4) * N + nb * NB + NB]
                        nc.tensor.matmul(reg, lhs4, rhs4, start=False, stop=True)
                ew = []
                for wop in range(2):
                    e = ev.tile([96, NB], f32, tag="ea", name="e3")[0:80]
                    nc.vector.reduce_max(e, p2[wop].rearrange("q (h n) -> q n h", h=2), axis=mybir.AxisListType.X)
                    ew.append(e)
                e4 = ev.tile([96, NB], f32, tag="ec", name="e4")[0:80]
                nc.vector.tensor_max(e4, ew[0], ew[1])
                nc.scalar.sign(
                    x3v[:, ho2 % 2, (ho2 // 2) * N + nb * NB : (ho2 // 2) * N + nb * NB + NB],
                    e4, bias=b2s,
                )

            def fc_block(nb):
                p3 = ps.tile([120, NB], f32, tag="ps")
                for h5p in range(2):
                    nc.tensor.matmul(
                        p3, f1drv[:, :, h5p * 128 : h5p * 128 + 120],
                        x3v[:, :, h5p * N + nb * NB : h5p * N + nb * NB + NB],
                        start=(h5p == 0), stop=False, perf_mode=DR,
                    )
                nc.tensor.matmul(
                    p3, f14s,
                    x3v[:, 0, 2 * N + nb * NB : 2 * N + nb * NB + NB],
                    start=False, stop=True,
                )
                nc.scalar.sign(x4[:, nb * NB : nb * NB + NB], p3, bias=bf1s)

                p4 = ps.tile([84, NB], f32, tag="ps", name="p1")
                nc.tensor.matmul(p4, f2s, x4[:, nb * NB : nb * NB + NB], start=True, stop=True)
                nc.scalar.sign(x5[:, nb * NB : nb * NB + NB], p4, bias=bf2s)

                p5 = ps.tile([10, NB], f32, tag="ps")
                nc.tensor.matmul(p5, f3s, x5[:, nb * NB : nb * NB + NB], start=True, stop=True)
                nc.vector.tensor_scalar_add(outs[:, nb * NB : nb * NB + NB], p5, bf3s)
                nc.sync.dma_start(
                    out=out_d[:, nb * NB : nb * NB + NB],
                    in_=outs[:, nb * NB : nb * NB + NB],
                )

            # ---- emission: conv1, with conv2's first blocks lag-interleaved
            # between conv1's last blocks so PSUM-pool reallocations always
            # land on buffers whose previous user's pool chain drained a full
            # block earlier (kills the conv1->conv2 boundary stall). The fc
            # chain is emitted right after conv2(4, nb) so only the last nb's
            # fc chain trails.
            early_c2 = [(0, 0), (0, 1), (1, 0), (1, 1)]
            sched = {(12, 0): [(0, 0)], (12, 1): [(0, 1)],
                     (13, 0): [(1, 0)], (13, 1): [(1, 1)]}
            for ho2 in range(14):
                for nb in range(NBLK):
                    p = [ps.tile([84, 2 * NB], f32, tag="ps", name="p1") for _ in range(2)]
                    conv1_hi(ho2, nb, p)
                    conv1_lo_pool(ho2, nb, p)
                    for c, cnb in sched.get((ho2, nb), []):
                        conv2_block(c, cnb)
            for c in range(2, 5):
                for nb in range(NBLK):
                    conv2_block(c, nb)
                    if c == 4:
                        fc_block(nb)

    nc.finalize()
    _CACHE["nc"] = nc
    return nc


def _install_ntff_hook():
    """The container's antenv stub lacks axon_hooks; synthesize it and register
    the ctypes-based NTFF profile hook from the axon boot module."""
    if "hook" in _CACHE:
        return
    _CACHE["hook"] = True
    try:
        import types
        import antenv

        if not hasattr(antenv, "axon_hooks"):
            store = {"h": None}
            m = types.ModuleType("antenv.axon_hooks")
            m.set_axon_ntff_profile_hook = lambda h: store.update(h=h)
            m.get_axon_ntff_profile_hook = lambda: store["h"]
            sys.modules["antenv.axon_hooks"] = m
            antenv.axon_hooks = m
            sys.path.insert(0, "/root/.axon_site")
            from trn_agent_boot.trn_boot import _ntff_profile_via_ctypes

            m.set_axon_ntff_profile_hook(
                _ntff_profile_via_ctypes("/opt/axon/libaxon_pjrt.so")
            )
    except Exception as e:  # profiling is best-effort
        print(f"ntff hook install failed: {e}", file=sys.stderr)


def kernel(x, w1, b1, w2, b2, wf1, bf1, wf2, bf2, wf3, bf3):
    nc = _get_nc()
    _install_ntff_hook()
    from concourse import bass_utils

    # host-side relayout: xt[core][c*32+w, h*N+n] = x[core*N+n, c, h, w]
    xr = np.ascontiguousarray(
        x.reshape(NCORES, N, 3, 32, 32).transpose(0, 2, 4, 3, 1)
    ).reshape(NCORES, 96, 32 * N)

    xh = xr.astype(np.float16)
    xl = ((xr - xh.astype(np.float32)) * 2048.0).astype(np.float16)

    t2dr, t24 = _build_t2(w2)
    f1dr, f14 = _build_f1(wf1)
    shared = {
        "t1h": _build_t1h(w1),
        "t1l": _build_t1l(w1),
        "t2dr": t2dr, "t24": t24, "f1dr": f1dr, "f14": f14,
        "f2": np.ascontiguousarray(_binarize(wf2).T).astype(BF16),
        "f3": np.ascontiguousarray(_binarize(wf3).T).astype(BF16),
        "b1v": np.tile(b1.astype(np.float32), 14).reshape(84, 1),
        "b2v": np.tile(b2.astype(np.float32), 5).reshape(80, 1),
        "bf1v": bf1.astype(np.float32).reshape(120, 1),
        "bf2v": bf2.astype(np.float32).reshape(84, 1),
        "bf3v": bf3.astype(np.float32).reshape(10, 1),
    }
    in_maps = [
        dict(shared, xh=np.ascontiguousarray(xh[i]), xl=np.ascontiguousarray(xl[i]))
        for i in range(NCORES)
    ]

    res = bass_utils.run_bass_kernel_spmd(
        nc, in_maps, core_ids=list(range(NCORES)),
        trace=bool(int(os.environ.get("KERNEL_TRACE", "0"))),
    )
    if res.exec_time_ns is not None:
        print(f"HW exec time: {res.exec_time_ns} ns")
    out = np.stack([r["out"] for r in res.results])  # [8, 10, N]
    return np.ascontiguousarray(out.transpose(0, 2, 1)).reshape(B, 10).astype(np.float32)
